# revision 1
# baseline (speedup 1.0000x reference)
"""Quantized 3x3 conv (int8-style QAT conv) on 8 TRN2 NeuronCores.

Reference semantics:
    qx = clip(round(x * (127/3)), -127, 127)          # int values in f32
    qw = clip(round(w * (127/0.05)), -127, 127)
    out = conv2d(qx, qw, stride 1, pad 1) * (3*0.05/127^2) + bias[None,:,None,None]

Strategy: pure data parallelism over batch (32 images -> 4 per core), no
collectives. Quantized values are integers <= 127, which are exact in bf16,
so the conv runs as bf16 matmuls with fp32 PSUM accumulation (bit-accurate
int arithmetic). Per core:
  - Activations are quantized on DVE only (tensor_scalar x3: mult+magic-add
    for round-to-nearest-even, then clamps) into zero-padded bf16 tiles,
    split into top/bottom halves per image so the first matmuls start after
    only half an image is resident. ScalarE must stay out of this chain:
    its FIFO is filled with epilogues and would stall the PE.
  - Weights ship as [tap, ci, co] (host does the pure layout permute), are
    DMA'd chunk-by-chunk and quantized via ScalarE round + GpSimd clamps.
  - The 3x3 conv = 9 shifted bf16 matmuls accumulated in PSUM. For each
    group of 4 row-tiles (8 rows x 56 cols, N=448 <= one PSUM bank) and
    each 128-wide cout chunk: out[co, y, x] += qw[tap][ci, co].T @
    qx[ci, y+dy, x+dx], weights loaded once per 4 matmuls.
  - ~5us of dummy matmuls bridge the input-pipeline head so the PE's HAM
    clock gate is already at 8/8 (2.4 GHz) when the real stream begins.
  - Epilogues (rescale + bias from PSUM) ride ScalarE so VectorE keeps its
    budget for quantization (the second-busiest engine at full PE rate); the
    final two groups alternate ScalarE/VectorE to parallelize the tail.
    Output DMAs use the ACT HWDGE ring, input loads the SP ring.
Measured steady state (paired-slope through the axon tunnel): ~45-55us/core
in unloaded windows, ~75us under moderate external load on the shared chip
(PE-stream bound either way; the 504 N=448 matmuls are gapless after an
~8.5us data-latency head, with a ~4.5us drain tail).
"""

import numpy as np

import concourse.mybir as mybir
import concourse.tile as tile
from concourse import bacc
from concourse.bass_utils import run_bass_kernel_spmd

# Problem constants
B, CIN, COUT, H, W, KS = 32, 128, 256, 56, 56, 3
NCORES = 8
BPC = B // NCORES          # images per core
NPIX = H * W               # 3136
HP = H + 2                 # padded spatial
QL = 127.0
SX = QL / 3.0              # activation quant scale
SW = QL / 0.05             # weight quant scale
RESCALE = (3.0 * 0.05) / (QL * QL)
MAGIC = 1.5 * 2.0**23      # fp32 round-to-nearest-even trick

ROWS = 8                   # output rows per matmul tile
RT = H // ROWS             # 7 row tiles per image
NTAP = KS * KS
NCHUNK = COUT // 128       # 2 cout chunks
GROUP = 4                  # psum tiles sharing one weight load

F32 = mybir.dt.float32
BF16 = mybir.dt.bfloat16

_NC = None


def _build(reps: int = 1, no_in: bool = False, no_out: bool = False,
           no_mm: bool = False, no_quant: bool = False, quant_mode: int = 5):
    """Build the SPMD graph. reps>1 wraps the whole per-call pipeline in a
    hardware For loop — used only by the timing harness (bench.py) to
    measure per-iteration HW time through the high-latency tunnel.
    no_in/no_out/no_mm ablate pipeline stages for bottleneck hunting."""
    nc = bacc.Bacc("TRN2", target_bir_lowering=False, num_devices=NCORES)

    x_t = nc.dram_tensor("x", [BPC, CIN, NPIX], F32, kind="ExternalInput")
    w_t = nc.dram_tensor("weight", [NTAP, CIN, COUT], F32, kind="ExternalInput")
    b_t = nc.dram_tensor("bias", [NCHUNK, 128, 1], F32, kind="ExternalInput")
    o_t = nc.dram_tensor("out", [BPC, NCHUNK, 128, NPIX], F32, kind="ExternalOutput")

    with tile.TileContext(nc) as tc:
        with (
            tc.tile_pool(name="consts", bufs=1) as consts,
            tc.tile_pool(name="xq", bufs=1) as xqp,
            tc.tile_pool(name="xstage", bufs=2) as xsp,
            tc.tile_pool(name="tmp", bufs=2) as tmpp,
            tc.tile_pool(name="outp", bufs=6) as outp,
            tc.tile_pool(name="psum", bufs=8, space="PSUM") as psp,
        ):
            # ---- padded quantized activations: top/bottom half tiles per
            # image. Split tiles give the matmuls finer-grained deps, so the
            # first groups start after only half an image is quantized. Only
            # the pad borders are memset (the interior is fully overwritten).
            # top tile = padded rows 0..33, bottom tile = padded rows 32..57.
            TROWS, BROWS = 34, 26
            xqt, xqb = [], []
            for b in range(BPC):
                tt = xqp.tile([128, TROWS, HP], BF16, tag=f"xqt{b}")
                bt = xqp.tile([128, BROWS, HP], BF16, tag=f"xqb{b}")
                nc.gpsimd.memset(tt[:, 0, :], 0.0)
                nc.gpsimd.memset(tt[:, 1:TROWS, 0], 0.0)
                nc.gpsimd.memset(tt[:, 1:TROWS, HP - 1], 0.0)
                nc.gpsimd.memset(bt[:, BROWS - 1, :], 0.0)
                nc.gpsimd.memset(bt[:, 0 : BROWS - 1, 0], 0.0)
                nc.gpsimd.memset(bt[:, 0 : BROWS - 1, HP - 1], 0.0)
                xqt.append(tt)
                xqb.append(bt)

            # ---- weights: DMA [ci, tap, co] on the ACT HWDGE ring (parallel
            # with x loads on the SP ring), quantize per cout-chunk: ACT does
            # the scale+round, GpSimd does the clamps so DVE stays free for
            # activation quant ----
            wq = consts.tile([128, NTAP, COUT], BF16, tag="wq")
            for c in range(NCHUNK):
                wraw = consts.tile([128, NTAP, 128], F32, tag=f"wraw{c}")
                weng = nc.sync if c == 0 else nc.scalar
                weng.dma_start(
                    out=wraw[:],
                    in_=w_t[:, :, c * 128:(c + 1) * 128].rearrange("t p c -> p t c"),
                )
                wtmp = consts.tile([128, NTAP, 128], F32, tag=f"wtmp{c}")
                nc.scalar.activation(
                    wtmp[:], wraw[:], mybir.ActivationFunctionType.Copy,
                    bias=MAGIC, scale=SW,
                )
                nc.gpsimd.tensor_scalar(
                    wtmp[:], wtmp[:], MAGIC, -QL,
                    mybir.AluOpType.subtract, mybir.AluOpType.max,
                )
                nc.gpsimd.tensor_scalar_min(
                    wq[:, :, c * 128:(c + 1) * 128], wtmp[:], QL)

            bias_sb = []
            for c in range(NCHUNK):
                bs = consts.tile([128, 1], F32, tag=f"bias{c}")
                nc.scalar.dma_start(out=bs[:], in_=b_t[c])
                bias_sb.append(bs)

            # ---- PE warmup: ~5us of dummy matmuls starting at t~0 flips the
            # HAM clock gate to 8/8 before the real matmuls begin (the PE is
            # idle during the input/weight pipelines anyway) ----
            warm = consts.tile([128, 512], BF16, tag="warm")
            nc.gpsimd.memset(warm[:], 1.0)
            wpt = psp.tile([128, 512], F32, tag="pt", name="warm_pt")
            for i in range(14):
                nc.tensor.matmul(wpt[:], warm[:, 0:128], warm[:, 0:512],
                                 start=True, stop=True)

            def body(_iv=None):
                # (x row0, nrows, dst list, dst row offset) for the halves:
                # top interior rows 1..33 <- x rows 0..32; bottom local rows
                # 0..24 <- x rows 31..55 (rows 31..32 quantized twice).
                halves = [(0, TROWS - 1, xqt, 1), (31, H - 31, xqb, 0)]
                for b in range(BPC) if not no_in else []:
                    for hi, (row0, nrows, dst_list, drow) in enumerate(halves):
                        xs = xsp.tile([128, nrows * W], F32, tag=f"xs{hi}",
                                      name=f"xs{b}_{hi}")
                        nc.sync.dma_start(
                            out=xs[:],
                            in_=x_t[b, :, row0 * W : (row0 + nrows) * W])
                        if no_quant:
                            continue
                        t1 = tmpp.tile([128, nrows * W], F32, tag=f"t1_{hi}",
                                       name=f"t1_{b}_{hi}")
                        nc.vector.tensor_scalar(
                            t1[:], xs[:], SX, MAGIC,
                            mybir.AluOpType.mult, mybir.AluOpType.add,
                        )
                        nc.vector.tensor_scalar(
                            t1[:], t1[:], MAGIC, -QL,
                            mybir.AluOpType.subtract, mybir.AluOpType.max,
                        )
                        nc.vector.tensor_scalar_min(
                            dst_list[b][:, drow : drow + nrows, 1 : W + 1],
                            t1[:].rearrange("p (h w) -> p h w", h=nrows),
                            QL,
                        )

                # ---- conv: 9 shifted matmuls accumulated in PSUM ----
                tiles = [] if no_mm else [(b, r) for b in range(BPC) for r in range(RT)]
                sizes = [GROUP] * (len(tiles) // GROUP - 1) + [GROUP - 1, 1] \
                    if tiles else []
                bounds = [0]
                for s in sizes:
                    bounds.append(bounds[-1] + s)
                for g in range(len(sizes)):
                    grp = tiles[bounds[g] : bounds[g + 1]]
                    for c in range(NCHUNK):
                        pts = [
                            psp.tile([128, ROWS * W], F32, tag="pt",
                                     name=f"pt{g}_{c}_{i}")
                            for i, _ in enumerate(grp)
                        ]
                        for tap in range(NTAP):
                            ky, kx = divmod(tap, KS)
                            lhsT = wq[:, tap, c * 128 : (c + 1) * 128]
                            for t, (b, r) in enumerate(grp):
                                prow = r * ROWS + ky
                                if r < 4:
                                    rhs = xqt[b][:, prow : prow + ROWS,
                                                 kx : kx + W]
                                else:
                                    rhs = xqb[b][:, prow - 32 : prow - 32 + ROWS,
                                                 kx : kx + W]
                                nc.tensor.matmul(
                                    pts[t][:], lhsT, rhs,
                                    start=(tap == 0), stop=(tap == NTAP - 1),
                                )
                        # epilogues ride ScalarE so VectorE keeps its budget
                        # for quantization (DVE is the second-busiest engine
                        # when the PE streams at full rate); the final two
                        # groups alternate ACT/DVE to parallelize the tail
                        tail_grp = g >= len(sizes) - 2
                        for t, (b, r) in enumerate(grp):
                            ot = outp.tile([128, ROWS * W], F32, tag="ot",
                                           name=f"ot{g}_{c}_{t}")
                            if not tail_grp or t % 2 == 0:
                                nc.scalar.activation(
                                    ot[:], pts[t][:],
                                    mybir.ActivationFunctionType.Identity,
                                    bias=bias_sb[c][:], scale=RESCALE,
                                )
                            else:
                                nc.vector.tensor_scalar(
                                    ot[:], pts[t][:], RESCALE, bias_sb[c][:],
                                    mybir.AluOpType.mult, mybir.AluOpType.add,
                                )
                            if not no_out:
                                nc.scalar.dma_start(
                                    out=o_t[b, c, :, r * ROWS * W : (r + 1) * ROWS * W],
                                    in_=ot[:],
                                )

            if reps == 1:
                body()
            else:
                with tc.For_i(0, reps, 1):
                    body()
    nc.compile()
    return nc


def _get_nc():
    global _NC
    if _NC is None:
        _NC = _build()
    return _NC


def kernel(x: np.ndarray, weight: np.ndarray, bias: np.ndarray) -> np.ndarray:
    """Full inputs in, full output out. Shards batch across 8 cores."""
    x = np.ascontiguousarray(np.asarray(x), dtype=np.float32).reshape(B, CIN, NPIX)
    # pure layout permute: [co, ci, ky, kx] -> [ky*kx, ci, co]
    w_l = np.ascontiguousarray(
        np.asarray(weight, dtype=np.float32).transpose(2, 3, 1, 0)
    ).reshape(NTAP, CIN, COUT)
    b_l = np.ascontiguousarray(
        np.asarray(bias, dtype=np.float32)).reshape(NCHUNK, 128, 1)

    nc = _get_nc()
    in_maps = [
        {
            "x": np.ascontiguousarray(x[i * BPC : (i + 1) * BPC]),
            "weight": w_l,
            "bias": b_l,
        }
        for i in range(NCORES)
    ]
    res = run_bass_kernel_spmd(nc, in_maps, core_ids=list(range(NCORES)))
    out = np.concatenate(
        [r["out"].reshape(BPC, COUT, H, W) for r in res.results], axis=0
    )
    return out

